# revision 12
# baseline (speedup 1.0000x reference)
"""Trainium2 Bass kernel for a binarized-weight MLP (BNN MNIST-style):

    h   = x @ sign(W1).T + b1      # fc1, binarized weights
    h   = clip(h, -1, 1)           # Hardtanh
    out = h @ W2.T + b2            # fc2

Shapes: x [8192, 784] f32, W1 [4096, 784], b1 [4096], W2 [10, 4096], b2 [10].

Strategy (data-parallel over 8 NeuronCores, v2):
  - Shard batch 8192 -> 1024 rows/core; replicate weights. All matmuls bf16
    (sign(W1) and the bias hi/lo split are exact in bf16), fp32 PSUM.
  - fc1 contraction is 784 x-lanes + 2 bias-lanes = 786 = 6 full k-tiles of
    128 plus an 18-lane tail. The 6 full tiles run as plain [128,512] MMs.
  - The 18-lane tail would waste a full 512-cycle MM per (ht, nt); instead
    4 tail MMs run CONCURRENTLY in one PE pass via row-group tiling
    (tile_position=(32j, 0)): the tail x-lanes are replicated into 4
    32-partition row groups of one moving tile, and each row group carries a
    different (ht, nt) tail with its own PSUM bank.
  - fc2 output is only 10 partitions; 4 fc2 MMs run concurrently via
    col-group tiling (tile_position=(0, 32j)), accumulating 4 bias-free
    partial bands in 2 PSUM banks (bands at partitions 0/32 for nt0 and
    64/96 for nt1). Bands are copied to SBUF, DMA'd out, summed on host
    (+ b2) -- host work is negligible.
  - Loop nest: 16 super-groups of (2 ht x 2 nt), plus a head group
    (ht0/ht1, nt0 only) and a tail group (ht0/ht1, nt1) so the DMA-starved
    start only needs the nt0 x-tiles. PSUM: 6-bank ps1 ring + 2 ps2 banks.
  - Per-core output: 4 bands [10, 512] f32 inside a [106, 512] DMA window;
    host gathers, sums band pairs, adds b2, transposes.
"""

import numpy as np
import ml_dtypes
from contextlib import ExitStack

import concourse.bass as bass
import concourse.mybir as mybir
import concourse.tile as tile
from concourse import bacc
from concourse import bass_utils

BF16_NP = ml_dtypes.bfloat16
F8_NP = ml_dtypes.float8_e4m3fn
BF16 = mybir.dt.bfloat16
F8 = mybir.dt.float8e4
F32 = mybir.dt.float32

BATCH, IN, HID, OUT = 8192, 784, 4096, 10
NCORES = 8
B_CORE = BATCH // NCORES        # 1024
NT = B_CORE // 512              # 2 batch n-tiles of 512 per core
HT = HID // 128                 # 32 hidden tiles
KT6 = 6                         # full 128-lane k-tiles (768 lanes)
K6 = KT6 * 128                  # 768
TAIL = IN - K6 + 2              # 18 tail lanes: x rows 768:784 + b1 hi/lo
NG = HT // 2                    # 16 super-groups of 2 ht
N_WARMUP = 8                    # PE warm-up matmuls: ~3.4us back-to-back,
                                # covers the whole HAM cold window + DGE
                                # ramp, so all real matmuls run at 2.4 GHz

_CACHE = {}


def _build():
    """Build + compile the Bacc graph once per process."""
    if "nc" in _CACHE:
        return _CACHE["nc"]

    nc = bacc.Bacc("TRN2", target_bir_lowering=False, debug=False,
                   num_devices=NCORES)
    xt_d = nc.dram_tensor("xt", [NT, KT6, 128, 512], BF16,
                          kind="ExternalInput").ap()
    x7_d = nc.dram_tensor("x7", [128, 512], BF16, kind="ExternalInput").ap()
    # sign(W1) in {-1,0,+1} and the fp8 hi/lo split of b1 are exact /
    # near-exact in fp8e4: halves the weight DMA bytes and doubles FWL
    # LDWEIGHTS speed; the PE upconverts operands, so numerics match bf16.
    w1_d = nc.dram_tensor("w1", [HT, 128, K6], F8,
                          kind="ExternalInput").ap()
    w1t_d = nc.dram_tensor("w1t", [NG, 128, 128], F8,
                           kind="ExternalInput").ap()
    w2_d = nc.dram_tensor("w2", [128, HT * OUT], BF16,
                          kind="ExternalInput").ap()
    out_d = nc.dram_tensor("out", [106, 512], F32,
                           kind="ExternalOutput").ap()

    # Raw SBUF tensor for PE warm-up matmuls: contents irrelevant
    # (uninitialized bf16 garbage), so the warm-ups have no producers.
    warm_sb = nc.alloc_sbuf_tensor("warm_raw", [128, 512], BF16).ap()

    with tile.TileContext(nc) as tc:
        with ExitStack() as ctx:
            wpool = ctx.enter_context(tc.tile_pool(name="w1", bufs=1))
            wtpool = ctx.enter_context(tc.tile_pool(name="w1t", bufs=1))
            xpool = ctx.enter_context(tc.tile_pool(name="x", bufs=1))
            cpool = ctx.enter_context(tc.tile_pool(name="const", bufs=1))
            hpool = ctx.enter_context(tc.tile_pool(name="h", bufs=2))
            ps1pool = ctx.enter_context(
                tc.tile_pool(name="ps1", bufs=6, space="PSUM"))
            ps2pool = ctx.enter_context(
                tc.tile_pool(name="ps2", bufs=1, space="PSUM"))

            w2_sb = cpool.tile([128, HT * OUT], BF16, tag="w2")
            out_sb = cpool.tile([128, 512], F32, tag="out")
            ps2 = [ps2pool.tile([128, 512], F32, tag=f"ps2_{nt}",
                                name=f"ps2_{nt}")
                   for nt in range(NT)]

            # PE warm-up: HAM clock gate keeps the PE at 1.2 GHz until
            # ~3.4us of sustained matmul activity; run dummy matmuls while
            # the first input DMAs stream. They write ps2 banks, whose band
            # partitions are later cleared by start=True fc2 matmuls.
            for i in range(N_WARMUP):
                nc.tensor.matmul(ps2[i % NT][:], warm_sb[:, 0:128],
                                 warm_sb[:], start=True, stop=True,
                                 skip_group_check=True)

            # ---- input DMAs on the two HWDGE queues (sync + scalar),
            # interleaved in consumption order. w1[0] is chunked so the
            # very first LDWEIGHTS only waits on a 32 KB transfer.
            w1_t = []
            x_t = {}

            def w1_dma(ht, chunks=None):
                t = wpool.tile([128, K6], F8, tag=f"w1_{ht}",
                               name=f"w1_{ht}")
                if chunks is None:
                    nc.sync.dma_start(t[:], w1_d[ht])
                else:
                    for a, b in chunks:
                        nc.sync.dma_start(t[:, a * 128:b * 128],
                                          w1_d[ht, :, a * 128:b * 128])
                w1_t.append(t)

            def x_dma(nt, kt, eng):
                t = xpool.tile([128, 512], BF16, tag=f"x_{nt}_{kt}",
                               name=f"x_{nt}_{kt}")
                eng.dma_start(t[:], xt_d[nt, kt])
                x_t[(nt, kt)] = t

            w1t_t = [wtpool.tile([128, 128], F8, tag=f"w1t_{t}",
                                 name=f"w1t_{t}") for t in range(NG)]
            x7_sb = xpool.tile([128, 512], BF16, tag="x7")

            # sync queue: w1[0] chunks interleaved with odd x tiles in
            # consumption order (each dma_start costs ~650ns of issue time
            # on the engine, so issue order = arrival order).
            w10 = wpool.tile([128, K6], F8, tag="w1_0", name="w1_0")
            w1_t.append(w10)

            def w10_chunk(a, b):
                nc.sync.dma_start(w10[:, a * 128:b * 128],
                                  w1_d[0, :, a * 128:b * 128])

            w10_chunk(0, 1)
            w1_dma(1, chunks=[(0, 1)])
            x_dma(0, 1, nc.sync)
            w10_chunk(1, 2)
            nc.sync.dma_start(w1_t[1][:, 128:768], w1_d[1, :, 128:768])
            x_dma(0, 3, nc.sync)
            w10_chunk(2, 4)
            x_dma(0, 5, nc.sync)
            w10_chunk(4, 6)
            nc.sync.dma_start(w1t_t[0][:], w1t_d[0])
            w1_dma(2)
            w1_dma(3)
            nc.sync.dma_start(w1t_t[1][:], w1t_d[1])
            x_dma(1, 1, nc.sync)
            x_dma(1, 3, nc.sync)
            x_dma(1, 5, nc.sync)
            for t in range(2, NG):
                w1_dma(2 * t)
                w1_dma(2 * t + 1)
                nc.sync.dma_start(w1t_t[t][:], w1t_d[t])
            # scalar queue: even x tiles, x7, w2
            x_dma(0, 0, nc.scalar)
            x_dma(0, 2, nc.scalar)
            x_dma(0, 4, nc.scalar)
            nc.scalar.dma_start(x7_sb[:], x7_d)
            x_dma(1, 0, nc.scalar)
            x_dma(1, 2, nc.scalar)
            x_dma(1, 4, nc.scalar)
            nc.scalar.dma_start(w2_sb[:], w2_d)

            # ---- main loop: head group H (ht0/ht1, nt0), 15 full groups,
            # tail group T (ht0/ht1, nt1).
            # groups: (t, ht_pair, nts)
            groups = [(0, (0, 1), (0,))]
            groups += [(t, (2 * t, 2 * t + 1), (0, 1)) for t in range(1, NG)]
            groups += [(0, (0, 1), (1,))]

            # fc2 band col-group per (ht parity, nt): nt0 -> 0/32,
            # nt1 -> 64/96.  Band (par, nt) accumulates over its group seq;
            # start on first, stop on last.
            def band_cg(par, nt):
                return 2 * nt + par              # col group index 0..3

            band_first = {}
            band_last = {}
            for gi, (t, hts, nts) in enumerate(groups):
                for par in range(2):
                    for nt in nts:
                        b = band_cg(par, nt)
                        if b not in band_first:
                            band_first[b] = gi
                        band_last[b] = gi

            pend = []          # delayed fc2 spans: (gi, [(ht, nt, h), ...])

            def flush_fc2(gitems):
                gi, items = gitems
                for ht, nt, h in items:
                    par = ht & 1
                    b = band_cg(par, nt)
                    p0 = 32 * b
                    nc.tensor.matmul(
                        ps2[nt][p0:p0 + OUT, :],
                        w2_sb[:, ht * OUT:(ht + 1) * OUT], h[:],
                        start=(gi == band_first[b]),
                        stop=(gi == band_last[b]),
                        skip_group_check=True,
                        tile_position=(0, p0))

            for gi, (t, hts, nts) in enumerate(groups):
                # fc1 core: 6 full k-tiles, stationary shared across nt.
                # Group H runs kt-major (both ht per x-tile) so the cold
                # start consumes x tiles no faster than the DMA ramp
                # delivers them -- keeps the PE busy so HAM un-throttles
                # on schedule.
                if gi == 0:
                    order = [(ht, kt) for kt in range(KT6) for ht in hts]
                else:
                    order = [(ht, kt) for ht in hts for kt in range(KT6)]
                ps1 = {}
                for ht, kt in order:
                    if True:
                        for nt in nts:
                            if (ht, nt) not in ps1:
                                ps1[(ht, nt)] = ps1pool.tile(
                                    [128, 512], F32, tag="ps1",
                                    name=f"ps1_{gi}_{ht}_{nt}")
                            nc.tensor.matmul(
                                ps1[(ht, nt)][:],
                                w1_t[ht][:, kt * 128:(kt + 1) * 128],
                                x_t[(nt, kt)][:],
                                start=(kt == 0), stop=False,
                                skip_group_check=True)
                # delayed fc2 span of the previous group runs here, while
                # this group's core MMs covered the previous group's DVE.
                if pend:
                    flush_fc2(pend.pop(0))
                # fc1 k-tail: concurrent row-group MMs, one per (ht, nt)
                for nt in nts:
                    for par, ht in enumerate(hts):
                        rg = 32 * band_cg(par, nt)
                        nc.tensor.matmul(
                            ps1[(ht, nt)][:],
                            w1t_t[t][rg:rg + TAIL, :],
                            x7_sb[rg:rg + TAIL, :],
                            start=False, stop=True,
                            skip_group_check=True,
                            tile_position=(rg, 0))
                # Hardtanh + downcast on DVE: h = max(min(ps1, 1), -1)
                items = []
                for nt in nts:
                    for par, ht in enumerate(hts):
                        h = hpool.tile([128, 512], BF16,
                                       tag=f"h_{band_cg(par, nt)}",
                                       name=f"h_{gi}_{ht}_{nt}")
                        nc.vector.tensor_scalar(
                            h[:], ps1[(ht, nt)][:], 1.0, -1.0,
                            op0=mybir.AluOpType.min,
                            op1=mybir.AluOpType.max)
                        items.append((ht, nt, h))
                pend.append((gi, items))
            while pend:
                flush_fc2(pend.pop(0))

            # ---- band eviction + output. nt0 bands (partitions 0:42)
            # complete one group before nt1 bands (64:106).
            nc.vector.tensor_copy(out_sb[0:42, :], ps2[0][0:42, :])
            nc.sync.dma_start(out_d[0:42, :], out_sb[0:42, :])
            nc.vector.tensor_copy(out_sb[64:106, :], ps2[1][64:106, :])
            nc.sync.dma_start(out_d[64:106, :], out_sb[64:106, :])

    nc.compile()
    _CACHE["nc"] = nc
    return nc


def _prep_inputs(x, W1, b1, W2, b2):
    """Host-side shard + layout prep. Returns in_maps for the 8 cores."""
    x = np.asarray(x, dtype=np.float32)
    W1 = np.asarray(W1, dtype=np.float32)
    b1 = np.asarray(b1, dtype=np.float32)
    W2 = np.asarray(W2, dtype=np.float32)

    # fc1 weight augmented with two bias rows (hi + lo fp8 split of b1):
    # rows 0:784 = sign(W1).T, row 784 = b1_hi, row 785 = b1_lo.
    # All values are exact (+-1/0) or a 2-term fp8 split, so fp8 storage
    # costs no accuracy vs bf16.
    w1aug = np.zeros((K6 + 128, HID), dtype=np.float32)
    w1aug[:IN] = np.sign(W1).T
    b1_hi = b1.astype(F8_NP).astype(np.float32)
    w1aug[IN] = b1_hi
    w1aug[IN + 1] = b1 - b1_hi
    w1aug8 = w1aug.astype(F8_NP)

    # main fc1 weight: [ht, p, kt*128+m] = w1aug[kt*128+p, ht*128+m]
    w1_host = np.ascontiguousarray(
        w1aug8[:K6].reshape(KT6, 128, HT, 128)
        .transpose(2, 1, 0, 3).reshape(HT, 128, K6))

    # tail weights: group t row-groups hold (ht=2t, 2t+1, 2t, 2t+1)
    w1t_host = np.zeros((NG, 128, 128), dtype=F8_NP)
    tail = w1aug8[K6:K6 + TAIL]                       # [18, 4096]
    for t in range(NG):
        for g in range(4):
            ht = 2 * t + (g & 1)
            w1t_host[t, 32 * g:32 * g + TAIL] = \
                tail[:, ht * 128:(ht + 1) * 128]
    w1t_host = np.ascontiguousarray(w1t_host)

    # fc2 weight: [p, ht*10+o] = W2[o, ht*128+p]
    w2_host = np.ascontiguousarray(
        W2.T.astype(BF16_NP).reshape(HT, 128, OUT)
        .transpose(1, 0, 2).reshape(128, HT * OUT))

    # x augmented with two ones-columns matching the b1 rows.
    x_aug = np.zeros((BATCH, K6 + TAIL), dtype=BF16_NP)
    x_aug[:, :IN] = x.astype(BF16_NP)
    x_aug[:, IN] = 1
    x_aug[:, IN + 1] = 1

    in_maps = []
    for c in range(NCORES):
        xc = x_aug[c * B_CORE:(c + 1) * B_CORE]          # [1024, 786]
        xt = np.ascontiguousarray(
            xc[:, :K6].reshape(NT, 512, KT6, 128).transpose(0, 2, 3, 1))
        x7 = np.zeros((128, 512), dtype=BF16_NP)
        for g in range(4):
            nt = g // 2
            x7[32 * g:32 * g + TAIL] = \
                xc[nt * 512:(nt + 1) * 512, K6:K6 + TAIL].T
        in_maps.append({"xt": xt, "x7": np.ascontiguousarray(x7),
                        "w1": w1_host, "w1t": w1t_host, "w2": w2_host})
    return in_maps


def _gather(results, b2):
    b2 = np.asarray(b2, dtype=np.float32)
    full = np.empty((BATCH, OUT), dtype=np.float32)
    for c, r in enumerate(results):
        d = np.asarray(r["out"], dtype=np.float32)       # [106, 512]
        nt0 = d[0:OUT] + d[32:32 + OUT]                  # [10, 512]
        nt1 = d[64:64 + OUT] + d[96:96 + OUT]
        full[c * B_CORE:c * B_CORE + 512] = nt0.T + b2
        full[c * B_CORE + 512:(c + 1) * B_CORE] = nt1.T + b2
    return full


def run(x, W1, b1, W2, b2, trace=False, **trace_kwargs):
    import os
    nc = _build()
    in_maps = _prep_inputs(x, W1, b1, W2, b2)
    if not trace:
        # The NTFF profiling hook isn't available in every environment;
        # make sure an ambient BASS_TRACE can't pull us onto that path.
        os.environ["BASS_NEVER_TRACE"] = "1"
    else:
        os.environ.pop("BASS_NEVER_TRACE", None)
    res = bass_utils.run_bass_kernel_spmd(
        nc, in_maps, core_ids=list(range(NCORES)), trace=trace,
        **trace_kwargs)
    return _gather(res.results, b2), res


def kernel(x, W1, b1, W2, b2):
    out, _ = run(x, W1, b1, W2, b2)
    return out


# revision 15
# speedup vs baseline: 1.0230x; 1.0230x over previous
"""Trainium2 Bass kernel for a binarized-weight MLP (BNN MNIST-style):

    h   = x @ sign(W1).T + b1      # fc1, binarized weights
    h   = clip(h, -1, 1)           # Hardtanh
    out = h @ W2.T + b2            # fc2

Shapes: x [8192, 784] f32, W1 [4096, 784], b1 [4096], W2 [10, 4096], b2 [10].

Strategy (data-parallel over 8 NeuronCores, v2):
  - Shard batch 8192 -> 1024 rows/core; replicate weights. All matmuls bf16
    (sign(W1) and the bias hi/lo split are exact in bf16), fp32 PSUM.
  - fc1 contraction is 784 x-lanes + 2 bias-lanes = 786 = 6 full k-tiles of
    128 plus an 18-lane tail. The 6 full tiles run as plain [128,512] MMs.
  - The 18-lane tail would waste a full 512-cycle MM per (ht, nt); instead
    4 tail MMs run CONCURRENTLY in one PE pass via row-group tiling
    (tile_position=(32j, 0)): the tail x-lanes are replicated into 4
    32-partition row groups of one moving tile, and each row group carries a
    different (ht, nt) tail with its own PSUM bank.
  - fc2 output is only 10 partitions; 4 fc2 MMs run concurrently via
    col-group tiling (tile_position=(0, 32j)), accumulating 4 bias-free
    partial bands in 2 PSUM banks (bands at partitions 0/32 for nt0 and
    64/96 for nt1). Bands are copied to SBUF, DMA'd out, summed on host
    (+ b2) -- host work is negligible.
  - Loop nest: 16 super-groups of (2 ht x 2 nt), plus a head group
    (ht0/ht1, nt0 only) and a tail group (ht0/ht1, nt1) so the DMA-starved
    start only needs the nt0 x-tiles. PSUM: 6-bank ps1 ring + 2 ps2 banks.
  - Per-core output: 4 bands [10, 512] f32 inside a [106, 512] DMA window;
    host gathers, sums band pairs, adds b2, transposes.
"""

import numpy as np
import ml_dtypes
from contextlib import ExitStack

import concourse.bass as bass
import concourse.mybir as mybir
import concourse.tile as tile
from concourse import bacc
from concourse import bass_utils

BF16_NP = ml_dtypes.bfloat16
F8_NP = ml_dtypes.float8_e4m3fn
BF16 = mybir.dt.bfloat16
F8 = mybir.dt.float8e4
F32 = mybir.dt.float32

BATCH, IN, HID, OUT = 8192, 784, 4096, 10
NCORES = 8
B_CORE = BATCH // NCORES        # 1024
NT = B_CORE // 512              # 2 batch n-tiles of 512 per core
HT = HID // 128                 # 32 hidden tiles
KT6 = 6                         # full 128-lane k-tiles (768 lanes)
K6 = KT6 * 128                  # 768
TAIL = IN - K6 + 2              # 18 tail lanes: x rows 768:784 + b1 hi/lo
NG = HT // 2                    # 16 super-groups of 2 ht
N_WARMUP = 8                    # PE warm-up matmuls: ~3.4us back-to-back,
                                # covers the whole HAM cold window + DGE
                                # ramp, so all real matmuls run at 2.4 GHz

_CACHE = {}


def _build():
    """Build + compile the Bacc graph once per process."""
    if "nc" in _CACHE:
        return _CACHE["nc"]

    nc = bacc.Bacc("TRN2", target_bir_lowering=False, debug=False,
                   num_devices=NCORES)
    xt_d = nc.dram_tensor("xt", [NT, KT6, 128, 512], BF16,
                          kind="ExternalInput").ap()
    x7_d = nc.dram_tensor("x7", [128, 512], BF16, kind="ExternalInput").ap()
    # sign(W1) in {-1,0,+1} and the fp8 hi/lo split of b1 are exact /
    # near-exact in fp8e4: halves the weight DMA bytes and doubles FWL
    # LDWEIGHTS speed; the PE upconverts operands, so numerics match bf16.
    w1_d = nc.dram_tensor("w1", [HT, 128, K6], F8,
                          kind="ExternalInput").ap()
    w1t_d = nc.dram_tensor("w1t", [NG, 128, 128], F8,
                           kind="ExternalInput").ap()
    w2_d = nc.dram_tensor("w2", [128, HT * OUT], BF16,
                          kind="ExternalInput").ap()
    out_d = nc.dram_tensor("out", [106, 512], F32,
                           kind="ExternalOutput").ap()

    # Raw SBUF tensor for PE warm-up matmuls: contents irrelevant
    # (uninitialized bf16 garbage), so the warm-ups have no producers.
    warm_sb = nc.alloc_sbuf_tensor("warm_raw", [128, 512], BF16).ap()

    with tile.TileContext(nc) as tc:
        with ExitStack() as ctx:
            wpool = ctx.enter_context(tc.tile_pool(name="w1", bufs=1))
            wtpool = ctx.enter_context(tc.tile_pool(name="w1t", bufs=1))
            xpool = ctx.enter_context(tc.tile_pool(name="x", bufs=1))
            cpool = ctx.enter_context(tc.tile_pool(name="const", bufs=1))
            hpool = ctx.enter_context(tc.tile_pool(name="h", bufs=2))
            ps1pool = ctx.enter_context(
                tc.tile_pool(name="ps1", bufs=6, space="PSUM"))
            ps2pool = ctx.enter_context(
                tc.tile_pool(name="ps2", bufs=1, space="PSUM"))

            w2_sb = cpool.tile([128, HT * OUT], BF16, tag="w2")
            out_sb = cpool.tile([128, 512], F32, tag="out")
            ps2 = [ps2pool.tile([128, 512], F32, tag=f"ps2_{nt}",
                                name=f"ps2_{nt}")
                   for nt in range(NT)]

            # PE warm-up: HAM clock gate keeps the PE at 1.2 GHz until
            # ~3.4us of sustained matmul activity; run dummy matmuls while
            # the first input DMAs stream. They write ps2 banks, whose band
            # partitions are later cleared by start=True fc2 matmuls.
            for i in range(N_WARMUP):
                nc.tensor.matmul(ps2[i % NT][:], warm_sb[:, 0:128],
                                 warm_sb[:], start=True, stop=True,
                                 skip_group_check=True)

            # ---- input DMAs on the two HWDGE queues (sync + scalar),
            # interleaved in consumption order. w1[0] is chunked so the
            # very first LDWEIGHTS only waits on a 32 KB transfer.
            w1_t = []
            x_t = {}

            def w1_dma(ht, chunks=None):
                t = wpool.tile([128, K6], F8, tag=f"w1_{ht}",
                               name=f"w1_{ht}")
                if chunks is None:
                    nc.sync.dma_start(t[:], w1_d[ht])
                else:
                    for a, b in chunks:
                        nc.sync.dma_start(t[:, a * 128:b * 128],
                                          w1_d[ht, :, a * 128:b * 128])
                w1_t.append(t)

            def x_dma(nt, kt, eng):
                t = xpool.tile([128, 512], BF16, tag=f"x_{nt}_{kt}",
                               name=f"x_{nt}_{kt}")
                eng.dma_start(t[:], xt_d[nt, kt])
                x_t[(nt, kt)] = t

            w1t_t = [wtpool.tile([128, 128], F8, tag=f"w1t_{t}",
                                 name=f"w1t_{t}") for t in range(NG)]
            x7_sb = xpool.tile([128, 512], BF16, tag="x7")

            # sync queue: w1[0] chunks interleaved with odd x tiles in
            # consumption order (each dma_start costs ~650ns of issue time
            # on the engine, so issue order = arrival order).
            w10 = wpool.tile([128, K6], F8, tag="w1_0", name="w1_0")
            w1_t.append(w10)

            def w10_chunk(a, b):
                nc.sync.dma_start(w10[:, a * 128:b * 128],
                                  w1_d[0, :, a * 128:b * 128])

            w10_chunk(0, 1)
            w1_dma(1, chunks=[(0, 1)])
            x_dma(0, 1, nc.sync)
            w10_chunk(1, 2)
            nc.sync.dma_start(w1_t[1][:, 128:768], w1_d[1, :, 128:768])
            x_dma(0, 3, nc.sync)
            w10_chunk(2, 4)
            x_dma(0, 5, nc.sync)
            w10_chunk(4, 6)
            nc.sync.dma_start(w1t_t[0][:], w1t_d[0])
            x_dma(1, 1, nc.sync)
            w1_dma(2)
            x_dma(1, 3, nc.sync)
            w1_dma(3)
            nc.sync.dma_start(w1t_t[1][:], w1t_d[1])
            x_dma(1, 5, nc.sync)
            for t in range(2, NG):
                w1_dma(2 * t)
                w1_dma(2 * t + 1)
                nc.sync.dma_start(w1t_t[t][:], w1t_d[t])
            # scalar queue: even x tiles, then nt1 evens interleaved early
            x_dma(0, 0, nc.scalar)
            x_dma(0, 2, nc.scalar)
            x_dma(0, 4, nc.scalar)
            x_dma(1, 0, nc.scalar)
            nc.scalar.dma_start(x7_sb[:], x7_d)
            x_dma(1, 2, nc.scalar)
            x_dma(1, 4, nc.scalar)
            nc.scalar.dma_start(w2_sb[:], w2_d)

            # ---- main loop: head group H (ht0/ht1, nt0), 15 full groups,
            # tail group T (ht0/ht1, nt1).
            # groups: (t, ht_pair, nts)
            groups = [(0, (0, 1), (0,))]
            groups += [(t, (2 * t, 2 * t + 1), (0, 1)) for t in range(1, NG)]
            groups += [(0, (0, 1), (1,))]

            # fc2 band col-group per (ht parity, nt): nt0 -> 0/32,
            # nt1 -> 64/96.  Band (par, nt) accumulates over its group seq;
            # start on first, stop on last.
            def band_cg(par, nt):
                return 2 * nt + par              # col group index 0..3

            band_first = {}
            band_last = {}
            for gi, (t, hts, nts) in enumerate(groups):
                for par in range(2):
                    for nt in nts:
                        b = band_cg(par, nt)
                        if b not in band_first:
                            band_first[b] = gi
                        band_last[b] = gi

            pend = []          # delayed fc2 spans: (gi, [(ht, nt, h), ...])

            def flush_fc2(gitems):
                gi, items = gitems
                for ht, nt, h in items:
                    par = ht & 1
                    b = band_cg(par, nt)
                    p0 = 32 * b
                    nc.tensor.matmul(
                        ps2[nt][p0:p0 + OUT, :],
                        w2_sb[:, ht * OUT:(ht + 1) * OUT], h[:],
                        start=(gi == band_first[b]),
                        stop=(gi == band_last[b]),
                        skip_group_check=True,
                        tile_position=(0, p0))

            for gi, (t, hts, nts) in enumerate(groups):
                # fc1 core: 6 full k-tiles, stationary shared across nt.
                # Group H runs kt-major (both ht per x-tile) so the cold
                # start consumes x tiles no faster than the DMA ramp
                # delivers them -- keeps the PE busy so HAM un-throttles
                # on schedule.
                if gi == 0:
                    # kt sequence matched to DMA arrival order: odd tiles
                    # come on the sync queue behind only ~32 KB of weight
                    # chunks, even tiles on the scalar queue.
                    order = [(ht, kt) for kt in (1, 0, 3, 2, 5, 4)
                             for ht in hts]
                else:
                    order = [(ht, kt) for ht in hts for kt in range(KT6)]
                ps1 = {}
                for ht, kt in order:
                    for nt in nts:
                        first = (ht, nt) not in ps1
                        if first:
                            ps1[(ht, nt)] = ps1pool.tile(
                                [128, 512], F32, tag="ps1",
                                name=f"ps1_{gi}_{ht}_{nt}")
                        nc.tensor.matmul(
                            ps1[(ht, nt)][:],
                            w1_t[ht][:, kt * 128:(kt + 1) * 128],
                            x_t[(nt, kt)][:],
                            start=first, stop=False,
                            skip_group_check=True)
                # delayed fc2 span of the previous group runs here, while
                # this group's core MMs covered the previous group's DVE.
                if pend:
                    flush_fc2(pend.pop(0))
                # fc1 k-tail: concurrent row-group MMs, one per (ht, nt)
                for nt in nts:
                    for par, ht in enumerate(hts):
                        rg = 32 * band_cg(par, nt)
                        nc.tensor.matmul(
                            ps1[(ht, nt)][:],
                            w1t_t[t][rg:rg + TAIL, :],
                            x7_sb[rg:rg + TAIL, :],
                            start=False, stop=True,
                            skip_group_check=True,
                            tile_position=(rg, 0))
                # Hardtanh + downcast on DVE: h = max(min(ps1, 1), -1)
                items = []
                for nt in nts:
                    for par, ht in enumerate(hts):
                        h = hpool.tile([128, 512], BF16,
                                       tag=f"h_{band_cg(par, nt)}",
                                       name=f"h_{gi}_{ht}_{nt}")
                        nc.vector.tensor_scalar(
                            h[:], ps1[(ht, nt)][:], 1.0, -1.0,
                            op0=mybir.AluOpType.min,
                            op1=mybir.AluOpType.max)
                        items.append((ht, nt, h))
                pend.append((gi, items))
            while pend:
                flush_fc2(pend.pop(0))

            # ---- band eviction + output. nt0 bands (partitions 0:42)
            # complete one group before nt1 bands (64:106).
            nc.vector.tensor_copy(out_sb[0:42, :], ps2[0][0:42, :])
            nc.sync.dma_start(out_d[0:42, :], out_sb[0:42, :])
            nc.vector.tensor_copy(out_sb[64:106, :], ps2[1][64:106, :])
            nc.sync.dma_start(out_d[64:106, :], out_sb[64:106, :])

    nc.compile()
    _CACHE["nc"] = nc
    return nc


def _prep_inputs(x, W1, b1, W2, b2):
    """Host-side shard + layout prep. Returns in_maps for the 8 cores."""
    x = np.asarray(x, dtype=np.float32)
    W1 = np.asarray(W1, dtype=np.float32)
    b1 = np.asarray(b1, dtype=np.float32)
    W2 = np.asarray(W2, dtype=np.float32)

    # fc1 weight augmented with two bias rows (hi + lo fp8 split of b1):
    # rows 0:784 = sign(W1).T, row 784 = b1_hi, row 785 = b1_lo.
    # All values are exact (+-1/0) or a 2-term fp8 split, so fp8 storage
    # costs no accuracy vs bf16.
    w1aug = np.zeros((K6 + 128, HID), dtype=np.float32)
    w1aug[:IN] = np.sign(W1).T
    b1_hi = b1.astype(F8_NP).astype(np.float32)
    w1aug[IN] = b1_hi
    w1aug[IN + 1] = b1 - b1_hi
    w1aug8 = w1aug.astype(F8_NP)

    # main fc1 weight: [ht, p, kt*128+m] = w1aug[kt*128+p, ht*128+m]
    w1_host = np.ascontiguousarray(
        w1aug8[:K6].reshape(KT6, 128, HT, 128)
        .transpose(2, 1, 0, 3).reshape(HT, 128, K6))

    # tail weights: group t row-groups hold (ht=2t, 2t+1, 2t, 2t+1)
    w1t_host = np.zeros((NG, 128, 128), dtype=F8_NP)
    tail = w1aug8[K6:K6 + TAIL]                       # [18, 4096]
    for t in range(NG):
        for g in range(4):
            ht = 2 * t + (g & 1)
            w1t_host[t, 32 * g:32 * g + TAIL] = \
                tail[:, ht * 128:(ht + 1) * 128]
    w1t_host = np.ascontiguousarray(w1t_host)

    # fc2 weight: [p, ht*10+o] = W2[o, ht*128+p]
    w2_host = np.ascontiguousarray(
        W2.T.astype(BF16_NP).reshape(HT, 128, OUT)
        .transpose(1, 0, 2).reshape(128, HT * OUT))

    # x augmented with two ones-columns matching the b1 rows.
    x_aug = np.zeros((BATCH, K6 + TAIL), dtype=BF16_NP)
    x_aug[:, :IN] = x.astype(BF16_NP)
    x_aug[:, IN] = 1
    x_aug[:, IN + 1] = 1

    in_maps = []
    for c in range(NCORES):
        xc = x_aug[c * B_CORE:(c + 1) * B_CORE]          # [1024, 786]
        xt = np.ascontiguousarray(
            xc[:, :K6].reshape(NT, 512, KT6, 128).transpose(0, 2, 3, 1))
        x7 = np.zeros((128, 512), dtype=BF16_NP)
        for g in range(4):
            nt = g // 2
            x7[32 * g:32 * g + TAIL] = \
                xc[nt * 512:(nt + 1) * 512, K6:K6 + TAIL].T
        in_maps.append({"xt": xt, "x7": np.ascontiguousarray(x7),
                        "w1": w1_host, "w1t": w1t_host, "w2": w2_host})
    return in_maps


def _gather(results, b2):
    b2 = np.asarray(b2, dtype=np.float32)
    full = np.empty((BATCH, OUT), dtype=np.float32)
    for c, r in enumerate(results):
        d = np.asarray(r["out"], dtype=np.float32)       # [106, 512]
        nt0 = d[0:OUT] + d[32:32 + OUT]                  # [10, 512]
        nt1 = d[64:64 + OUT] + d[96:96 + OUT]
        full[c * B_CORE:c * B_CORE + 512] = nt0.T + b2
        full[c * B_CORE + 512:(c + 1) * B_CORE] = nt1.T + b2
    return full


def run(x, W1, b1, W2, b2, trace=False, **trace_kwargs):
    import os
    nc = _build()
    in_maps = _prep_inputs(x, W1, b1, W2, b2)
    if not trace:
        # The NTFF profiling hook isn't available in every environment;
        # make sure an ambient BASS_TRACE can't pull us onto that path.
        os.environ["BASS_NEVER_TRACE"] = "1"
    else:
        os.environ.pop("BASS_NEVER_TRACE", None)
    res = bass_utils.run_bass_kernel_spmd(
        nc, in_maps, core_ids=list(range(NCORES)), trace=trace,
        **trace_kwargs)
    return _gather(res.results, b2), res


def kernel(x, W1, b1, W2, b2):
    out, _ = run(x, W1, b1, W2, b2)
    return out


# revision 16
# speedup vs baseline: 1.0260x; 1.0029x over previous
"""Trainium2 Bass kernel for a binarized-weight MLP (BNN MNIST-style):

    h   = x @ sign(W1).T + b1      # fc1, binarized weights
    h   = clip(h, -1, 1)           # Hardtanh
    out = h @ W2.T + b2            # fc2

Shapes: x [8192, 784] f32, W1 [4096, 784], b1 [4096], W2 [10, 4096], b2 [10].

Strategy (data-parallel over 8 NeuronCores, v2):
  - Shard batch 8192 -> 1024 rows/core; replicate weights. All matmuls bf16
    (sign(W1) and the bias hi/lo split are exact in bf16), fp32 PSUM.
  - fc1 contraction is 784 x-lanes + 2 bias-lanes = 786 = 6 full k-tiles of
    128 plus an 18-lane tail. The 6 full tiles run as plain [128,512] MMs.
  - The 18-lane tail would waste a full 512-cycle MM per (ht, nt); instead
    4 tail MMs run CONCURRENTLY in one PE pass via row-group tiling
    (tile_position=(32j, 0)): the tail x-lanes are replicated into 4
    32-partition row groups of one moving tile, and each row group carries a
    different (ht, nt) tail with its own PSUM bank.
  - fc2 output is only 10 partitions; 4 fc2 MMs run concurrently via
    col-group tiling (tile_position=(0, 32j)), accumulating 4 bias-free
    partial bands in 2 PSUM banks (bands at partitions 0/32 for nt0 and
    64/96 for nt1). Bands are copied to SBUF, DMA'd out, summed on host
    (+ b2) -- host work is negligible.
  - Loop nest: 16 super-groups of (2 ht x 2 nt), plus a head group
    (ht0/ht1, nt0 only) and a tail group (ht0/ht1, nt1) so the DMA-starved
    start only needs the nt0 x-tiles. PSUM: 6-bank ps1 ring + 2 ps2 banks.
  - Per-core output: 4 bands [10, 512] f32 inside a [106, 512] DMA window;
    host gathers, sums band pairs, adds b2, transposes.
"""

import numpy as np
import ml_dtypes
from contextlib import ExitStack

import concourse.bass as bass
import concourse.mybir as mybir
import concourse.tile as tile
from concourse import bacc
from concourse import bass_utils

BF16_NP = ml_dtypes.bfloat16
F8_NP = ml_dtypes.float8_e4m3fn
BF16 = mybir.dt.bfloat16
F8 = mybir.dt.float8e4
F32 = mybir.dt.float32

BATCH, IN, HID, OUT = 8192, 784, 4096, 10
NCORES = 8
B_CORE = BATCH // NCORES        # 1024
NT = B_CORE // 512              # 2 batch n-tiles of 512 per core
HT = HID // 128                 # 32 hidden tiles
KT6 = 6                         # full 128-lane k-tiles (768 lanes)
K6 = KT6 * 128                  # 768
TAIL = IN - K6 + 2              # 18 tail lanes: x rows 768:784 + b1 hi/lo
NG = HT // 2                    # 16 super-groups of 2 ht
N_WARMUP = 8                    # PE warm-up matmuls: ~3.4us back-to-back,
                                # covers the whole HAM cold window + DGE
                                # ramp, so all real matmuls run at 2.4 GHz

_CACHE = {}


def _build():
    """Build + compile the Bacc graph once per process."""
    if "nc" in _CACHE:
        return _CACHE["nc"]

    nc = bacc.Bacc("TRN2", target_bir_lowering=False, debug=False,
                   num_devices=NCORES)
    xt_d = nc.dram_tensor("xt", [NT, KT6, 128, 512], BF16,
                          kind="ExternalInput").ap()
    x7_d = nc.dram_tensor("x7", [128, 512], BF16, kind="ExternalInput").ap()
    # sign(W1) in {-1,0,+1} and the fp8 hi/lo split of b1 are exact /
    # near-exact in fp8e4: halves the weight DMA bytes and doubles FWL
    # LDWEIGHTS speed; the PE upconverts operands, so numerics match bf16.
    w1_d = nc.dram_tensor("w1", [HT, 128, K6], F8,
                          kind="ExternalInput").ap()
    w1t_d = nc.dram_tensor("w1t", [NG, 128, 128], F8,
                           kind="ExternalInput").ap()
    w2_d = nc.dram_tensor("w2", [128, HT * OUT], BF16,
                          kind="ExternalInput").ap()
    out_d = nc.dram_tensor("out", [106, 512], F32,
                           kind="ExternalOutput").ap()

    # Raw SBUF tensor for PE warm-up matmuls: contents irrelevant
    # (uninitialized bf16 garbage), so the warm-ups have no producers.
    warm_sb = nc.alloc_sbuf_tensor("warm_raw", [128, 512], BF16).ap()

    with tile.TileContext(nc) as tc:
        with ExitStack() as ctx:
            wpool = ctx.enter_context(tc.tile_pool(name="w1", bufs=1))
            wtpool = ctx.enter_context(tc.tile_pool(name="w1t", bufs=1))
            xpool = ctx.enter_context(tc.tile_pool(name="x", bufs=1))
            cpool = ctx.enter_context(tc.tile_pool(name="const", bufs=1))
            hpool = ctx.enter_context(tc.tile_pool(name="h", bufs=2))
            ps1pool = ctx.enter_context(
                tc.tile_pool(name="ps1", bufs=6, space="PSUM"))
            ps2pool = ctx.enter_context(
                tc.tile_pool(name="ps2", bufs=1, space="PSUM"))

            w2_sb = cpool.tile([128, HT * OUT], BF16, tag="w2")
            out_sb = cpool.tile([128, 512], F32, tag="out")
            ps2 = [ps2pool.tile([128, 512], F32, tag=f"ps2_{nt}",
                                name=f"ps2_{nt}")
                   for nt in range(NT)]

            # PE warm-up: HAM clock gate keeps the PE at 1.2 GHz until
            # ~3.4us of sustained matmul activity; run dummy matmuls while
            # the first input DMAs stream. They write ps2 banks, whose band
            # partitions are later cleared by start=True fc2 matmuls.
            for i in range(N_WARMUP):
                nc.tensor.matmul(ps2[i % NT][:], warm_sb[:, 0:128],
                                 warm_sb[:], start=True, stop=True,
                                 skip_group_check=True)

            # ---- input DMAs on the two HWDGE queues (sync + scalar),
            # interleaved in consumption order. w1[0] is chunked so the
            # very first LDWEIGHTS only waits on a 32 KB transfer.
            w1_t = []
            x_t = {}

            def w1_dma(ht, chunks=None):
                t = wpool.tile([128, K6], F8, tag=f"w1_{ht}",
                               name=f"w1_{ht}")
                if chunks is None:
                    nc.sync.dma_start(t[:], w1_d[ht])
                else:
                    for a, b in chunks:
                        nc.sync.dma_start(t[:, a * 128:b * 128],
                                          w1_d[ht, :, a * 128:b * 128])
                w1_t.append(t)

            def x_dma(nt, kt, eng):
                t = xpool.tile([128, 512], BF16, tag=f"x_{nt}_{kt}",
                               name=f"x_{nt}_{kt}")
                eng.dma_start(t[:], xt_d[nt, kt])
                x_t[(nt, kt)] = t

            w1t_t = [wtpool.tile([128, 128], F8, tag=f"w1t_{t}",
                                 name=f"w1t_{t}") for t in range(NG)]
            x7_sb = xpool.tile([128, 512], BF16, tag="x7")

            # sync queue: w1[0] chunks interleaved with odd x tiles in
            # consumption order (each dma_start costs ~650ns of issue time
            # on the engine, so issue order = arrival order).
            w10 = wpool.tile([128, K6], F8, tag="w1_0", name="w1_0")
            w1_t.append(w10)

            def w10_chunk(a, b):
                nc.sync.dma_start(w10[:, a * 128:b * 128],
                                  w1_d[0, :, a * 128:b * 128])

            w10_chunk(0, 1)
            w1_dma(1, chunks=[(0, 1)])
            x_dma(0, 1, nc.sync)
            w10_chunk(1, 2)
            nc.sync.dma_start(w1_t[1][:, 128:768], w1_d[1, :, 128:768])
            x_dma(0, 3, nc.sync)
            w10_chunk(2, 4)
            x_dma(0, 5, nc.sync)
            w10_chunk(4, 6)
            nc.sync.dma_start(w1t_t[0][:], w1t_d[0])
            x_dma(1, 1, nc.sync)
            w1_dma(2)
            x_dma(1, 3, nc.sync)
            w1_dma(3)
            nc.sync.dma_start(w1t_t[1][:], w1t_d[1])
            x_dma(1, 5, nc.sync)
            for t in range(2, NG):
                w1_dma(2 * t)
                w1_dma(2 * t + 1)
                nc.sync.dma_start(w1t_t[t][:], w1t_d[t])
            # scalar queue: even x tiles, then nt1 evens interleaved early
            x_dma(0, 0, nc.scalar)
            x_dma(0, 2, nc.scalar)
            x_dma(0, 4, nc.scalar)
            x_dma(1, 0, nc.scalar)
            nc.scalar.dma_start(x7_sb[:], x7_d)
            x_dma(1, 2, nc.scalar)
            x_dma(1, 4, nc.scalar)
            nc.scalar.dma_start(w2_sb[:], w2_d)

            # ---- main loop: head group H (ht0/ht1, nt0), 15 full groups,
            # tail group T (ht0/ht1, nt1).
            # groups: (t, ht_pair, nts)
            groups = [(0, (0, 1), (0,))]
            groups += [(t, (2 * t, 2 * t + 1), (0, 1)) for t in range(1, NG)]
            groups += [(0, (0, 1), (1,))]

            # fc2 band col-group per (ht parity, nt): nt0 -> 0/32,
            # nt1 -> 64/96.  Band (par, nt) accumulates over its group seq;
            # start on first, stop on last.
            def band_cg(par, nt):
                return 2 * nt + par              # col group index 0..3

            band_first = {}
            band_last = {}
            for gi, (t, hts, nts) in enumerate(groups):
                for par in range(2):
                    for nt in nts:
                        b = band_cg(par, nt)
                        if b not in band_first:
                            band_first[b] = gi
                        band_last[b] = gi

            pend = []          # delayed fc2 spans: (gi, [(ht, nt, h), ...])

            def flush_fc2(gitems):
                gi, items = gitems
                for ht, nt, h in items:
                    par = ht & 1
                    b = band_cg(par, nt)
                    p0 = 32 * b
                    nc.tensor.matmul(
                        ps2[nt][p0:p0 + OUT, :],
                        w2_sb[:, ht * OUT:(ht + 1) * OUT], h[:],
                        start=(gi == band_first[b]),
                        stop=(gi == band_last[b]),
                        skip_group_check=True,
                        tile_position=(0, p0))

            for gi, (t, hts, nts) in enumerate(groups):
                # fc1 core: 6 full k-tiles, stationary shared across nt.
                # Group H runs kt-major (both ht per x-tile) so the cold
                # start consumes x tiles no faster than the DMA ramp
                # delivers them -- keeps the PE busy so HAM un-throttles
                # on schedule.
                if gi == 0:
                    # kt sequence matched to DMA arrival order: odd tiles
                    # come on the sync queue behind only ~32 KB of weight
                    # chunks, even tiles on the scalar queue.
                    order = [(ht, kt) for kt in (1, 0, 3, 2, 5, 4)
                             for ht in hts]
                else:
                    order = [(ht, kt) for ht in hts for kt in range(KT6)]
                ps1 = {}
                for qi, (ht, kt) in enumerate(order):
                    for nt in nts:
                        first = (ht, nt) not in ps1
                        if first:
                            ps1[(ht, nt)] = ps1pool.tile(
                                [128, 512], F32, tag="ps1",
                                name=f"ps1_{gi}_{ht}_{nt}")
                        nc.tensor.matmul(
                            ps1[(ht, nt)][:],
                            w1_t[ht][:, kt * 128:(kt + 1) * 128],
                            x_t[(nt, kt)][:],
                            start=first, stop=False,
                            skip_group_check=True)
                    if gi == 0 and ht == hts[-1] and kt != order[-1][1]:
                        # head group runs at the DMA ramp's pace: after each
                        # kt pair, a dep-free filler matmul keeps the PE busy
                        # through the next x-tile's arrival so the HAM clock
                        # gate stays un-throttled.
                        nfill = 2 if qi == 1 else 1
                        for _ in range(nfill):
                            nc.tensor.matmul(ps2[0][:], warm_sb[:, 0:128],
                                             warm_sb[:], start=True,
                                             stop=True,
                                             skip_group_check=True)
                # delayed fc2 span of the previous group runs here, while
                # this group's core MMs covered the previous group's DVE.
                if pend:
                    flush_fc2(pend.pop(0))
                # fc1 k-tail: concurrent row-group MMs, one per (ht, nt)
                for nt in nts:
                    for par, ht in enumerate(hts):
                        rg = 32 * band_cg(par, nt)
                        nc.tensor.matmul(
                            ps1[(ht, nt)][:],
                            w1t_t[t][rg:rg + TAIL, :],
                            x7_sb[rg:rg + TAIL, :],
                            start=False, stop=True,
                            skip_group_check=True,
                            tile_position=(rg, 0))
                # Hardtanh + downcast on DVE: h = max(min(ps1, 1), -1)
                items = []
                for nt in nts:
                    for par, ht in enumerate(hts):
                        h = hpool.tile([128, 512], BF16,
                                       tag=f"h_{band_cg(par, nt)}",
                                       name=f"h_{gi}_{ht}_{nt}")
                        nc.vector.tensor_scalar(
                            h[:], ps1[(ht, nt)][:], 1.0, -1.0,
                            op0=mybir.AluOpType.min,
                            op1=mybir.AluOpType.max)
                        items.append((ht, nt, h))
                pend.append((gi, items))
            while pend:
                flush_fc2(pend.pop(0))

            # ---- band eviction + output. nt0 bands (partitions 0:42)
            # complete one group before nt1 bands (64:106).
            nc.vector.tensor_copy(out_sb[0:42, :], ps2[0][0:42, :])
            nc.sync.dma_start(out_d[0:42, :], out_sb[0:42, :])
            nc.vector.tensor_copy(out_sb[64:106, :], ps2[1][64:106, :])
            nc.sync.dma_start(out_d[64:106, :], out_sb[64:106, :])

    nc.compile()
    _CACHE["nc"] = nc
    return nc


def _prep_inputs(x, W1, b1, W2, b2):
    """Host-side shard + layout prep. Returns in_maps for the 8 cores."""
    x = np.asarray(x, dtype=np.float32)
    W1 = np.asarray(W1, dtype=np.float32)
    b1 = np.asarray(b1, dtype=np.float32)
    W2 = np.asarray(W2, dtype=np.float32)

    # fc1 weight augmented with two bias rows (hi + lo fp8 split of b1):
    # rows 0:784 = sign(W1).T, row 784 = b1_hi, row 785 = b1_lo.
    # All values are exact (+-1/0) or a 2-term fp8 split, so fp8 storage
    # costs no accuracy vs bf16.
    w1aug = np.zeros((K6 + 128, HID), dtype=np.float32)
    w1aug[:IN] = np.sign(W1).T
    b1_hi = b1.astype(F8_NP).astype(np.float32)
    w1aug[IN] = b1_hi
    w1aug[IN + 1] = b1 - b1_hi
    w1aug8 = w1aug.astype(F8_NP)

    # main fc1 weight: [ht, p, kt*128+m] = w1aug[kt*128+p, ht*128+m]
    w1_host = np.ascontiguousarray(
        w1aug8[:K6].reshape(KT6, 128, HT, 128)
        .transpose(2, 1, 0, 3).reshape(HT, 128, K6))

    # tail weights: group t row-groups hold (ht=2t, 2t+1, 2t, 2t+1)
    w1t_host = np.zeros((NG, 128, 128), dtype=F8_NP)
    tail = w1aug8[K6:K6 + TAIL]                       # [18, 4096]
    for t in range(NG):
        for g in range(4):
            ht = 2 * t + (g & 1)
            w1t_host[t, 32 * g:32 * g + TAIL] = \
                tail[:, ht * 128:(ht + 1) * 128]
    w1t_host = np.ascontiguousarray(w1t_host)

    # fc2 weight: [p, ht*10+o] = W2[o, ht*128+p]
    w2_host = np.ascontiguousarray(
        W2.T.astype(BF16_NP).reshape(HT, 128, OUT)
        .transpose(1, 0, 2).reshape(128, HT * OUT))

    # x augmented with two ones-columns matching the b1 rows.
    x_aug = np.zeros((BATCH, K6 + TAIL), dtype=BF16_NP)
    x_aug[:, :IN] = x.astype(BF16_NP)
    x_aug[:, IN] = 1
    x_aug[:, IN + 1] = 1

    in_maps = []
    for c in range(NCORES):
        xc = x_aug[c * B_CORE:(c + 1) * B_CORE]          # [1024, 786]
        xt = np.ascontiguousarray(
            xc[:, :K6].reshape(NT, 512, KT6, 128).transpose(0, 2, 3, 1))
        x7 = np.zeros((128, 512), dtype=BF16_NP)
        for g in range(4):
            nt = g // 2
            x7[32 * g:32 * g + TAIL] = \
                xc[nt * 512:(nt + 1) * 512, K6:K6 + TAIL].T
        in_maps.append({"xt": xt, "x7": np.ascontiguousarray(x7),
                        "w1": w1_host, "w1t": w1t_host, "w2": w2_host})
    return in_maps


def _gather(results, b2):
    b2 = np.asarray(b2, dtype=np.float32)
    full = np.empty((BATCH, OUT), dtype=np.float32)
    for c, r in enumerate(results):
        d = np.asarray(r["out"], dtype=np.float32)       # [106, 512]
        nt0 = d[0:OUT] + d[32:32 + OUT]                  # [10, 512]
        nt1 = d[64:64 + OUT] + d[96:96 + OUT]
        full[c * B_CORE:c * B_CORE + 512] = nt0.T + b2
        full[c * B_CORE + 512:(c + 1) * B_CORE] = nt1.T + b2
    return full


def run(x, W1, b1, W2, b2, trace=False, **trace_kwargs):
    import os
    nc = _build()
    in_maps = _prep_inputs(x, W1, b1, W2, b2)
    if not trace:
        # The NTFF profiling hook isn't available in every environment;
        # make sure an ambient BASS_TRACE can't pull us onto that path.
        os.environ["BASS_NEVER_TRACE"] = "1"
    else:
        os.environ.pop("BASS_NEVER_TRACE", None)
    res = bass_utils.run_bass_kernel_spmd(
        nc, in_maps, core_ids=list(range(NCORES)), trace=trace,
        **trace_kwargs)
    return _gather(res.results, b2), res


def kernel(x, W1, b1, W2, b2):
    out, _ = run(x, W1, b1, W2, b2)
    return out
